# revision 14
# baseline (speedup 1.0000x reference)
"""BoxBlur 7x7 (normalized, reflect padding) on 8 Trainium2 NeuronCores.

Strategy (pure data parallel, 4 images x 3 channels = 12 image-planes per core):
  - Horizontal 7-tap box sum per 128-row tile on VectorE:
    one fp32 prefix scan (tensor_tensor_scan) along the free dim over a
    reflect-padded row buffer, then one shifted subtract S[c+7]-S[c].
  - Vertical 7-tap weighted sum as a banded matmul on TensorE:
    out_tile[122, 1024] = W[K,122].T @ h[K, 1024], where the band matrix W
    carries the 1/49 normalization and the vertical reflect folding at the
    image top/bottom.  Output tiles are 122 rows so K = 122+6 = 128 fits the
    PE array contraction dim exactly.
  - PSUM evacuated to SBUF on ScalarE; input loads alternate between the SP
    and ACT HWDGE rings; output stores go through the gpsimd SWDGE queue,
    which spreads packets across all 16 SDMA engines (HWDGE sbuf->dram
    lands on only 2 engines and caps at ~50 GB/s).
"""

import numpy as np

import concourse.bass as bass
import concourse.tile as tile
from concourse import bacc, mybir
from concourse.bass_utils import run_bass_kernel_spmd

H = W = 1024
KH = KW = 7
PAD = 3            # k // 2
OUT_TILE = 122     # 128 - 2*PAD, so K = OUT_TILE + 2*PAD = 128
N_TILES = 9        # 8 * 122 = 976, last tile has 48 rows
N_CORES = 8
IMGS_PER_CORE = 4  # 32 / 8
CHANNELS = 3
IC_PER_CORE = IMGS_PER_CORE * CHANNELS      # 12 image-planes per core
ROWS = IC_PER_CORE * H                      # 12288

# padded row buffer: col 0 = 0.0, cols 1..3 = left reflect, 4..1027 = x,
# 1028..1030 = right reflect.  S[c] = prefix sum of cols 1..c.
PBUF = W + KW        # 1031 valid columns
PBUF_ALLOC = 1032    # round to 8B

_F32 = mybir.dt.float32

# DMA queue assignment experiments: each dma_start is issued from one of
# these engines (round-robin per tile).  'sync'/'scalar' are HWDGE rings,
# 'gpsimd' is the SWDGE ring.
LOAD_ENGINES = ["sync", "scalar"]  # input loads alternate across both HWDGE rings
STORE_ENGINES = ["gpsimd"]         # SWDGE spreads stores across all 16 SDMA engines
STORE_MODE = "plain"
BUFS = 10
HPASS = "scan1"       # fused rolling-window scan: h[c] = h[c-1] + p[c+7] - p[c]
MM_DTYPE = "f32"      # exact fp32 matmul (f32r is ~2x PE speed, 5e-5 rel err)
LOAD_SPLIT = False    # split each load into two half-width DMAs on both rings

_compiled = None  # cached compiled Bass program


def _dma_eng(nc, names, i):
    return getattr(nc, names[i % len(names)])


def _tile_geometry():
    geo = []
    for t in range(N_TILES):
        out_lo = t * OUT_TILE
        out_hi = min(H, out_lo + OUT_TILE)
        k_lo = max(0, out_lo - PAD)
        k_hi = min(H, out_hi + PAD)
        geo.append((out_lo, out_hi, k_lo, k_hi))
    return geo


def _build_weights(kcol):
    """Band matrices for the vertical pass, one per distinct tile kind.

    kcol: [7] fp32 per-tap vertical weights (already includes full 2D
    normalization; horizontal pass is an unweighted 7-tap sum)."""
    ws = []
    for (out_lo, out_hi, k_lo, k_hi) in _tile_geometry():
        K = k_hi - k_lo
        M = out_hi - out_lo
        Wm = np.zeros((K, M), np.float32)
        for m in range(M):
            R = out_lo + m
            for d in range(-PAD, PAD + 1):
                r = R + d
                if r < 0:
                    r = -r
                if r > H - 1:
                    r = 2 * (H - 1) - r
                Wm[r - k_lo, m] += kcol[d + PAD]
        ws.append(Wm)
    # tiles 1..7 share identical geometry/weights
    return ws[0], ws[1], ws[-1]


def _body(tc, nc, x, w0, wm, w8, out, n_ic=IC_PER_CORE):
    geo = _tile_geometry()
    f32 = _F32
    mm_dt = mybir.dt.float32r if MM_DTYPE == "f32r" else _F32
    with (
        tc.tile_pool(name="wpool", bufs=1) as wpool,
        tc.tile_pool(name="xpad", bufs=BUFS) as xpool,
        tc.tile_pool(name="scan", bufs=BUFS) as spool,
        tc.tile_pool(name="hbuf", bufs=BUFS) as hpool,
        tc.tile_pool(name="psum", bufs=4, space="PSUM") as ppool,
        tc.tile_pool(name="osb", bufs=BUFS) as opool,
    ):
        w0_t = wpool.tile([128, OUT_TILE], mm_dt, tag="w0")
        nc.sync.dma_start(w0_t[0:125, :], w0)
        wm_t = wpool.tile([128, OUT_TILE], mm_dt, tag="wm")
        nc.sync.dma_start(wm_t[:, :], wm)
        w8_t = wpool.tile([128, 48], mm_dt, tag="w8")
        nc.sync.dma_start(w8_t[0:51, :], w8)

        for ic in range(n_ic):
            rbase = ic * H
            for t in range(N_TILES):
                out_lo, out_hi, k_lo, k_hi = geo[t]
                P = k_hi - k_lo
                M = out_hi - out_lo
                if t == 0:
                    wt = w0_t[0:P, 0:M]
                elif t == N_TILES - 1:
                    wt = w8_t[0:P, 0:M]
                else:
                    wt = wm_t[0:P, 0:M]

                tile_idx = ic * N_TILES + t
                xp = xpool.tile([128, PBUF_ALLOC], f32, tag="xp")
                if LOAD_SPLIT:
                    hw = W // 2
                    _dma_eng(nc, LOAD_ENGINES, 2 * tile_idx).dma_start(
                        xp[0:P, KW - PAD : KW - PAD + hw],
                        x[rbase + k_lo : rbase + k_hi, 0:hw],
                    )
                    _dma_eng(nc, LOAD_ENGINES, 2 * tile_idx + 1).dma_start(
                        xp[0:P, KW - PAD + hw : KW - PAD + W],
                        x[rbase + k_lo : rbase + k_hi, hw:W],
                    )
                else:
                    _dma_eng(nc, LOAD_ENGINES, tile_idx).dma_start(
                        xp[0:P, KW - PAD : KW - PAD + W],
                        x[rbase + k_lo : rbase + k_hi, :],
                    )
                nc.vector.memset(xp[0:P, 0:1], 0.0)
                # left reflect: cols 1..3 = x[3],x[2],x[1] = buf cols 7,6,5
                nc.vector.tensor_copy(xp[0:P, 1:4], xp[0:P, 7:4:-1])
                # right reflect: cols 1028..1030 = x[1022..1020] = buf 1026,1025,1024
                nc.vector.tensor_copy(xp[0:P, 1028:1031], xp[0:P, 1026:1023:-1])

                h = hpool.tile([128, W], mm_dt, tag="h")
                if HPASS == "scan1":
                    # rolling window: state[c] = state[c-1] + p[c+7] - p[c],
                    # initial = sum(p[0:7])
                    init = spool.tile([128, 8], f32, tag="S")
                    nc.vector.tensor_reduce(
                        init[0:P, 0:1], xp[0:P, 0:KW],
                        axis=mybir.AxisListType.X, op=mybir.AluOpType.add,
                    )
                    nc.vector.tensor_tensor_scan(
                        h[0:P, :],
                        xp[0:P, KW:PBUF],
                        xp[0:P, 0:W],
                        init[0:P, 0:1],
                        op0=mybir.AluOpType.add,
                        op1=mybir.AluOpType.subtract,
                    )
                else:
                    S = spool.tile([128, PBUF_ALLOC], f32, tag="S")
                    nc.vector.tensor_tensor_scan(
                        S[0:P, 0:PBUF],
                        xp[0:P, 0:PBUF],
                        xp[0:P, 0:PBUF],
                        0.0,
                        op0=mybir.AluOpType.add,
                        op1=mybir.AluOpType.bypass,
                    )
                    nc.vector.tensor_sub(h[0:P, :], S[0:P, KW:PBUF], S[0:P, 0:W])

                ps = ppool.tile([128, W], f32, tag="ps")
                for half in range(2):
                    sl = slice(half * 512, (half + 1) * 512)
                    nc.tensor.matmul(
                        ps[0:M, sl], wt, h[0:P, sl], start=True, stop=True
                    )

                if STORE_MODE == "offset":
                    ob = opool.tile([128, W + 8], f32, tag="ob")
                    obv = ob[0:M, 4 : 4 + W]
                else:
                    ob = opool.tile([128, W], f32, tag="ob")
                    obv = ob[0:M, :]
                nc.scalar.copy(obv, ps[0:M, :])
                if STORE_MODE == "strided":
                    eng = _dma_eng(nc, STORE_ENGINES, tile_idx)
                    eng.dma_start(
                        out[rbase + out_lo : rbase + out_hi : 2, :], ob[0:M:2, :]
                    )
                    eng.dma_start(
                        out[rbase + out_lo + 1 : rbase + out_hi : 2, :], ob[1:M:2, :]
                    )
                else:
                    _dma_eng(nc, STORE_ENGINES, tile_idx).dma_start(
                        out[rbase + out_lo : rbase + out_hi, :], obv
                    )


def _bass_program(n_ic=IC_PER_CORE, num_devices=N_CORES):
    nc = bacc.Bacc(
        "TRN2",
        target_bir_lowering=False,
        debug=False,
        enable_asserts=False,
        num_devices=num_devices,
    )
    rows = n_ic * H
    mm_dt = mybir.dt.float32r if MM_DTYPE == "f32r" else _F32
    x_ap = nc.dram_tensor("x", [rows, W], _F32, kind="ExternalInput").ap()
    w0_ap = nc.dram_tensor("w0", [125, OUT_TILE], mm_dt, kind="ExternalInput").ap()
    wm_ap = nc.dram_tensor("wm", [128, OUT_TILE], mm_dt, kind="ExternalInput").ap()
    w8_ap = nc.dram_tensor("w8", [51, 48], mm_dt, kind="ExternalInput").ap()
    out_ap = nc.dram_tensor("out", [rows, W], _F32, kind="ExternalOutput").ap()
    with tile.TileContext(nc) as tc:
        _body(tc, nc, x_ap, w0_ap, wm_ap, w8_ap, out_ap, n_ic=n_ic)
    nc.compile()
    return nc


def _get_program():
    global _compiled
    if _compiled is None:
        _compiled = _bass_program()
    return _compiled


def _make_in_maps(x, kernel):
    x = np.ascontiguousarray(np.asarray(x, dtype=np.float32))
    assert x.shape == (N_CORES * IMGS_PER_CORE, CHANNELS, H, W), x.shape
    k2 = np.asarray(kernel, dtype=np.float64)
    k2 = k2 / k2.sum()
    # horizontal pass is an unweighted 7-tap sum => all columns of the
    # normalized kernel must be identical (true for the box kernel).
    assert np.allclose(k2, k2[:, :1]), "kernel must have uniform rows"
    kcol = k2[:, 0].astype(np.float32)
    w0, wm, w8 = _build_weights(kcol)
    xr = x.reshape(N_CORES, ROWS, W)
    return [
        {"x": xr[c], "w0": w0, "wm": wm, "w8": w8} for c in range(N_CORES)
    ]


def run_shards(in_maps, **kwargs):
    """Compile (cached) + run on cores 0..7; returns BassKernelResults."""
    nc = _get_program()
    return run_bass_kernel_spmd(nc, in_maps, core_ids=list(range(N_CORES)), **kwargs)


def kernel(x, kernel):
    in_maps = _make_in_maps(x, kernel)
    try:
        res = run_shards(in_maps)
    except Exception:
        # one retry: transient NRT device errors have been observed under
        # the PJRT/axon path; the device recovers on a fresh dispatch.
        import time as _time

        _time.sleep(30)
        res = run_shards(in_maps)
    outs = [
        res.results[c]["out"].reshape(IMGS_PER_CORE, CHANNELS, H, W)
        for c in range(N_CORES)
    ]
    return np.concatenate(outs, axis=0)
